# Initial kernel scaffold
#
"""DU-LM-SB Trainium2 kernel.

Host: LM setup (J, h, c0) in numpy (one-time, ~0.5 GFLOP).
Device (8 NeuronCores, batch-sharded 512/core): T-step SB recurrence.

Per step t (all per-step scalars baked as immediates at build time):
    PSUM = J@x + [h] + cx*x + cv*v          (PE: fp32r matmuls, diag trick)
    xs   = dka*PSUM (+ dka*h via ACT bias)  -> x' (position update)
    s1, s2 = Silu(xs+1), Silu(xs-1)         (ACT)
    xn   = (s1 - 1) - s2                    = phi_s(x')  (DVE, f32r out)
    axn  = |xn|                             (DVE int-and)
    mp   = 1 + tanh(50.5 - 50*axn)          = 2*(1 - psi_s(xn))  (ACT + DVE)
    W    = x - xs                           = -dk*y'   (GPSIMD)
    v    = mp*W                             (DVE/GPSIMD)
with yv_t = -v_{t-1}/(2*dk_{t-1}) folded into the cv coefficient.
"""
import math

import numpy as np

NCORES = 8
NRB = 1024          # N*rb = 2*Nt*rb
BSZ = 4096
BSH = BSZ // NCORES  # 512 batch per core
NPAIR = 4            # 8 row-blocks of 128, paired into [128, 1024] tiles

# tuning flags
E1_VARIANT = "act"       # "act": ACT copy w/ h bias + carry via diag MM
                         # "stt": DVE STT w/ exact fp32 x carry + h via PE
E9_GPS_PAIRS = (1, 3)    # which pairs run the final v=mp*W product on GPSIMD

_BUILD_CACHE = {}


def _lm_setup(H_real, H_imag, y_real, y_imag, lam, nbps):
    M = 2 ** nbps
    Nr, Nt = H_real.shape
    N = 2 * Nt
    rb = nbps // 2
    qam_var = 2.0 * (M - 1) / 3.0
    I_N = np.eye(N, dtype=np.float32)
    w = (2.0 ** (rb - 1 - np.arange(rb))).astype(np.float32)
    Tm = (w[:, None, None] * I_N[None]).reshape(-1, N).T
    H_tilde = np.block([[H_real, -H_imag], [H_imag, H_real]]).astype(np.float32)
    y_tilde = np.concatenate([y_real, y_imag], axis=0).astype(np.float32)
    G = (H_tilde @ H_tilde.T + lam * I_N).astype(np.float32)
    U = (np.linalg.inv(G) / lam).astype(np.float32)
    HT = (H_tilde @ Tm).astype(np.float32)
    J = (-(HT.T @ (U @ HT)) * np.float32(2.0 / qam_var)).astype(np.float32)
    J = (J * (1.0 - np.eye(J.shape[0], dtype=J.dtype))).astype(np.float32)
    z = ((y_tilde - HT @ np.ones((N * rb, 1), np.float32)
          + np.float32(math.sqrt(M) - 1.0) * (H_tilde @ np.ones((N, 1), np.float32)))
         / np.float32(math.sqrt(qam_var))).astype(np.float32)
    h = (2.0 * HT.T @ (U @ z)).astype(np.float32)
    c0 = np.float32(0.5 * math.sqrt(J.shape[0] - 1) / np.linalg.norm(J))
    return J, h, c0


def _build(T, dka, cx, cv, scl_hb):
    """Build + bacc-compile the per-core Bass program (all cores identical)."""
    import concourse.bacc as bacc
    import concourse.mybir as mybir
    import concourse.tile as tile

    dt = mybir.dt
    Alu = mybir.AluOpType
    Act = mybir.ActivationFunctionType

    nc = bacc.Bacc(None, target_bir_lowering=False, debug=False)
    jt_d = nc.dram_tensor("jt", [NRB, NRB], dt.float32r, kind="ExternalInput")
    x_d = nc.dram_tensor("x0", [NRB, BSH], dt.float32r, kind="ExternalInput")
    v_d = nc.dram_tensor("v0", [NRB, BSH], dt.float32r, kind="ExternalInput")
    hcol_d = nc.dram_tensor("hcol", [128, 8], dt.float32, kind="ExternalInput")
    hrow_d = nc.dram_tensor("hrow", [1, NRB], dt.float32r, kind="ExternalInput")
    eye_d = nc.dram_tensor("eye", [128, 128], dt.float32, kind="ExternalInput")
    out_d = nc.dram_tensor("xout", [NRB, BSH], dt.float32, kind="ExternalOutput")

    with tile.TileContext(nc) as tc:
        with (
            tc.tile_pool(name="jt", bufs=1) as jt_pool,
            tc.tile_pool(name="state", bufs=2) as state_pool,
            tc.tile_pool(name="tmp", bufs=2) as tmp_pool,
            tc.tile_pool(name="diag", bufs=2) as diag_pool,
            tc.tile_pool(name="ps", bufs=1, space="PSUM") as psum,
        ):
            jts = []
            for k in range(8):
                t_ = jt_pool.tile([128, NRB], dt.float32r, tag=f"jt{k}")
                nc.sync.dma_start(t_[:], jt_d.ap()[k * 128:(k + 1) * 128, :])
                jts.append(t_)
            hcol = jt_pool.tile([128, 8], dt.float32, tag="hcol")
            nc.sync.dma_start(hcol[:], hcol_d.ap())
            hrow = jt_pool.tile([1, NRB], dt.float32r, tag="hrow")
            nc.sync.dma_start(hrow[:], hrow_d.ap())
            eye = jt_pool.tile([128, 128], dt.float32, tag="eye")
            nc.sync.dma_start(eye[:], eye_d.ap())
            ones_f = jt_pool.tile([1, BSH], dt.float32, tag="ones")
            nc.vector.memset(ones_f[:], 1.0)
            b_p1 = jt_pool.tile([128, 1], dt.float32, tag="bp1")
            nc.vector.memset(b_p1[:], 1.0)
            b_m1 = jt_pool.tile([128, 1], dt.float32, tag="bm1")
            nc.vector.memset(b_m1[:], -1.0)
            b_th = jt_pool.tile([128, 1], dt.float32, tag="bth")
            nc.vector.memset(b_th[:], 50.5)

            xp, vp = [], []
            for jj in range(NPAIR):
                xt = state_pool.tile([128, 1024], dt.float32r, tag=f"x{jj}")
                nc.sync.dma_start(xt[:, 0:BSH],
                                  x_d.ap()[(2 * jj) * 128:(2 * jj + 1) * 128, :])
                nc.sync.dma_start(xt[:, BSH:2 * BSH],
                                  x_d.ap()[(2 * jj + 1) * 128:(2 * jj + 2) * 128, :])
                xp.append(xt)
                vt = state_pool.tile([128, 1024], dt.float32r, tag=f"v{jj}")
                nc.sync.dma_start(vt[:, 0:BSH],
                                  v_d.ap()[(2 * jj) * 128:(2 * jj + 1) * 128, :])
                nc.sync.dma_start(vt[:, BSH:2 * BSH],
                                  v_d.ap()[(2 * jj + 1) * 128:(2 * jj + 2) * 128, :])
                vp.append(vt)

            for t in range(T):
                dgx = diag_pool.tile([128, 128], dt.float32r, tag="dgx")
                nc.vector.tensor_scalar(dgx[:], eye[:], float(cx[t]), None, Alu.mult)
                dgv = diag_pool.tile([128, 128], dt.float32r, tag="dgv")
                nc.vector.tensor_scalar(dgv[:], eye[:], float(cv[t]), None, Alu.mult)
                if E1_VARIANT == "act":
                    hb = diag_pool.tile([128, 8], dt.float32, tag="hb")
                    nc.vector.tensor_scalar(hb[:], hcol[:], float(scl_hb[t]),
                                            None, Alu.mult)

                newx, newv = [], []
                for jj in range(NPAIR):
                    acc = psum.tile([128, 2 * BSH], dt.float32, tag=f"ps{jj}")
                    for half in range(2):
                        m = 2 * jj + half
                        sl = slice(half * BSH, (half + 1) * BSH)
                        first = True
                        if E1_VARIANT == "stt":
                            nc.tensor.matmul(acc[:, sl],
                                             hrow[:, m * 128:(m + 1) * 128],
                                             ones_f[:].bitcast(dt.float32r),
                                             start=True, stop=False)
                            first = False
                        for k in range(8):
                            nc.tensor.matmul(
                                acc[:, sl], jts[k][:, m * 128:(m + 1) * 128],
                                xp[k // 2][:, (k % 2) * BSH:(k % 2 + 1) * BSH],
                                start=first, stop=False)
                            first = False
                        nc.tensor.matmul(acc[:, sl], dgx[:], xp[jj][:, sl],
                                         start=False, stop=False)
                        nc.tensor.matmul(acc[:, sl], dgv[:], vp[jj][:, sl],
                                         start=False, stop=True)
                    x = xp[jj]
                    xs = tmp_pool.tile([128, 1024], dt.float32, tag="xs")
                    if E1_VARIANT == "act":
                        for half in range(2):
                            m = 2 * jj + half
                            sl = slice(half * BSH, (half + 1) * BSH)
                            nc.scalar.activation(xs[:, sl], acc[:, sl], Act.Identity,
                                                 bias=hb[:, m:m + 1],
                                                 scale=float(dka[t]))
                    else:
                        nc.vector.scalar_tensor_tensor(
                            xs[:], acc[:], float(dka[t]),
                            x[:].bitcast(dt.float32), Alu.mult, Alu.add)
                    s1 = tmp_pool.tile([128, 1024], dt.float32, tag="s1")
                    nc.scalar.activation(s1[:], xs[:], Act.Silu, bias=b_p1[:, :])
                    s2 = tmp_pool.tile([128, 1024], dt.float32, tag="s2")
                    nc.scalar.activation(s2[:], xs[:], Act.Silu, bias=b_m1[:, :])
                    xn = state_pool.tile([128, 1024], dt.float32r, tag=f"x{jj}")
                    nc.vector.scalar_tensor_tensor(
                        xn[:], s1[:], -1.0, s2[:], Alu.add, Alu.subtract)
                    newx.append(xn)
                    if t == T - 1:
                        newv.append(vp[jj])
                        continue
                    axn = tmp_pool.tile([128, 1024], dt.float32, tag="axn")
                    nc.vector.tensor_scalar(
                        axn[:].bitcast(dt.int32), xn[:].bitcast(dt.int32),
                        0x7FFFFFFF, None, Alu.bitwise_and)
                    mpp = tmp_pool.tile([128, 1024], dt.float32, tag="mpp")
                    nc.scalar.activation(mpp[:], axn[:], Act.Tanh,
                                         bias=b_th[:, :], scale=-50.0)
                    W = tmp_pool.tile([128, 1024], dt.float32, tag="W")
                    nc.gpsimd.tensor_tensor(
                        W[:], x[:].bitcast(dt.float32), xs[:], Alu.subtract)
                    mp = tmp_pool.tile([128, 1024], dt.float32, tag="mp")
                    nc.vector.tensor_scalar(mp[:], mpp[:], 1.0, None, Alu.add)
                    vn = state_pool.tile([128, 1024], dt.float32r, tag=f"v{jj}")
                    eng = nc.gpsimd if jj in E9_GPS_PAIRS else nc.vector
                    eng.tensor_tensor(vn[:], mp[:], W[:], Alu.mult)
                    newv.append(vn)
                xp, vp = newx, newv

            for jj in range(NPAIR):
                nc.sync.dma_start(out_d.ap()[(2 * jj) * 128:(2 * jj + 1) * 128, :],
                                  xp[jj][:, 0:BSH].bitcast(dt.float32))
                nc.sync.dma_start(out_d.ap()[(2 * jj + 1) * 128:(2 * jj + 2) * 128, :],
                                  xp[jj][:, BSH:2 * BSH].bitcast(dt.float32))
    nc.compile()
    return nc


def kernel(**inputs):
    H_real = np.asarray(inputs["H_real"], np.float32)
    H_imag = np.asarray(inputs["H_imag"], np.float32)
    y_real = np.asarray(inputs["y_real"], np.float32)
    y_imag = np.asarray(inputs["y_imag"], np.float32)
    Delta = np.asarray(inputs["Delta"], np.float32)
    eta = np.asarray(inputs["eta"], np.float32)
    lam = np.asarray(inputs["lam"], np.float32)
    x0 = np.asarray(inputs["x0"], np.float32)
    y0 = np.asarray(inputs["y0"], np.float32)
    nbps = int(np.asarray(inputs["nbps"]))

    J, h, c0 = _lm_setup(H_real, H_imag, y_real, y_imag, lam, nbps)

    T = Delta.shape[0]
    dk = Delta.astype(np.float64)
    a_sched = np.linspace(0.0, 1.0, T).astype(np.float32).astype(np.float64)
    eta0 = float(eta[0])
    alpha = dk * eta0 * float(c0)
    beta = dk * (1.0 - a_sched)
    dka = dk * alpha
    if E1_VARIANT == "act":
        cx = (1.0 - dk * beta) / (dk * alpha)
    else:
        cx = -beta / alpha
    cv = np.empty(T)
    cv[0] = 1.0 / alpha[0]
    for t in range(1, T):
        cv[t] = -1.0 / (2.0 * dk[t - 1] * alpha[t])
    scl_hb = dka  # bias scale for h in E1

    key = (T, E1_VARIANT, E9_GPS_PAIRS,
           dka.tobytes(), cx.tobytes(), cv.tobytes())
    if key not in _BUILD_CACHE:
        _BUILD_CACHE[key] = _build(T, dka, cx, cv, scl_hb)
    nc = _BUILD_CACHE[key]

    JT = np.ascontiguousarray(J.T)
    x_init = (0.02 * (x0 - 0.5)).astype(np.float32)
    v_init = (0.02 * (y0 - 0.5)).astype(np.float32)
    hcol = np.ascontiguousarray(h.reshape(8, 128).T)  # [128, 8] per-block h
    hrow = np.ascontiguousarray(h.reshape(1, NRB))
    eye = np.eye(128, dtype=np.float32)

    in_maps = []
    for c in range(NCORES):
        sl = slice(c * BSH, (c + 1) * BSH)
        in_maps.append({
            "jt": JT,
            "x0": np.ascontiguousarray(x_init[:, sl]),
            "v0": np.ascontiguousarray(v_init[:, sl]),
            "hcol": hcol,
            "hrow": hrow,
            "eye": eye,
        })

    from concourse.bass_utils import run_bass_kernel_spmd
    res = run_bass_kernel_spmd(nc, in_maps, core_ids=list(range(NCORES)),
                               trace=False)
    out = np.concatenate([r["xout"] for r in res.results], axis=1)  # [1024, 4096]
    return np.ascontiguousarray(out.T).astype(np.float32)


# revision 14
# speedup vs baseline: 1.3395x; 1.3395x over previous
"""DU-LM-SB Trainium2 kernel.

Host: LM setup (J, h, c0) in numpy (one-time, ~0.5 GFLOP).
Device (8 NeuronCores, batch-sharded 512/core): T-step SB recurrence.

Per step t (all per-step scalars baked as immediates at build time):
    PSUM = J@x + [h] + cx*x + cv*v          (PE: fp32r matmuls, diag trick)
    xs   = dka*PSUM (+ dka*h via ACT bias)  -> x' (position update)
    s1, s2 = Silu(xs+1), Silu(xs-1)         (ACT)
    xn   = (s1 - 1) - s2                    = phi_s(x')  (DVE, f32r out)
    axn  = |xn|                             (DVE int-and)
    mp   = 1 + tanh(50.5 - 50*axn)          = 2*(1 - psi_s(xn))  (ACT + DVE)
    W    = x - xs                           = -dk*y'   (GPSIMD)
    v    = mp*W                             (DVE/GPSIMD)
with yv_t = -v_{t-1}/(2*dk_{t-1}) folded into the cv coefficient.
"""
import math

import numpy as np

NCORES = 8
NRB = 1024          # N*rb = 2*Nt*rb
BSZ = 4096
BSH = BSZ // NCORES  # 512 batch per core
NPAIR = 4            # 8 row-blocks of 128, paired into [128, 1024] tiles

# tuning flags
E1_VARIANT = "act"       # "act": ACT copy w/ h bias + carry via diag MM
                         # "stt": DVE STT w/ exact fp32 x carry + h via PE
E9_GPS_PAIRS = ()    # which pairs run the final v=mp*W product on GPSIMD
W_GPS_PAIRS = ()  # W engine per pair
KMAJOR_HALVES = False     # interleave the two psum halves k-major in PE order
FINE_PAIRS = ()      # pairs with half-granular xn chains
QUAD_MASK = False         # quad-granular abs/tanh/mp in yv path

_BUILD_CACHE = {}


def _lm_setup(H_real, H_imag, y_real, y_imag, lam, nbps):
    M = 2 ** nbps
    Nr, Nt = H_real.shape
    N = 2 * Nt
    rb = nbps // 2
    qam_var = 2.0 * (M - 1) / 3.0
    I_N = np.eye(N, dtype=np.float32)
    w = (2.0 ** (rb - 1 - np.arange(rb))).astype(np.float32)
    Tm = (w[:, None, None] * I_N[None]).reshape(-1, N).T
    H_tilde = np.block([[H_real, -H_imag], [H_imag, H_real]]).astype(np.float32)
    y_tilde = np.concatenate([y_real, y_imag], axis=0).astype(np.float32)
    G = (H_tilde @ H_tilde.T + lam * I_N).astype(np.float32)
    U = (np.linalg.inv(G) / lam).astype(np.float32)
    HT = (H_tilde @ Tm).astype(np.float32)
    J = (-(HT.T @ (U @ HT)) * np.float32(2.0 / qam_var)).astype(np.float32)
    J = (J * (1.0 - np.eye(J.shape[0], dtype=J.dtype))).astype(np.float32)
    z = ((y_tilde - HT @ np.ones((N * rb, 1), np.float32)
          + np.float32(math.sqrt(M) - 1.0) * (H_tilde @ np.ones((N, 1), np.float32)))
         / np.float32(math.sqrt(qam_var))).astype(np.float32)
    h = (2.0 * HT.T @ (U @ z)).astype(np.float32)
    c0 = np.float32(0.5 * math.sqrt(J.shape[0] - 1) / np.linalg.norm(J))
    return J, h, c0


def _build(T, dka, cx, cv, scl_hb):
    """Build + bacc-compile the per-core Bass program (all cores identical)."""
    import concourse.bacc as bacc
    import concourse.mybir as mybir
    import concourse.tile as tile
    from concourse.tile import add_dep_helper

    dt = mybir.dt
    Alu = mybir.AluOpType
    Act = mybir.ActivationFunctionType

    nc = bacc.Bacc(None, target_bir_lowering=False, debug=False)
    jt_d = nc.dram_tensor("jt", [NRB, NRB], dt.float32r, kind="ExternalInput")
    x_d = nc.dram_tensor("x0", [NRB, BSH], dt.float32r, kind="ExternalInput")
    v_d = nc.dram_tensor("v0", [NRB, BSH], dt.float32r, kind="ExternalInput")
    hcol_d = nc.dram_tensor("hcol", [128, 8], dt.float32, kind="ExternalInput")
    hrow_d = nc.dram_tensor("hrow", [1, NRB], dt.float32r, kind="ExternalInput")
    eye_d = nc.dram_tensor("eye", [128, 128], dt.float32, kind="ExternalInput")
    out_d = nc.dram_tensor("xout", [NRB, BSH], dt.float32, kind="ExternalOutput")

    with tile.TileContext(nc) as tc:
        with (
            tc.tile_pool(name="jt", bufs=1) as jt_pool,
            tc.tile_pool(name="state", bufs=2) as state_pool,
            tc.tile_pool(name="tmp", bufs=2) as tmp_pool,
            tc.tile_pool(name="tmpq", bufs=1) as tmpq_pool,
            tc.tile_pool(name="diag", bufs=2) as diag_pool,
            tc.tile_pool(name="ps", bufs=1, space="PSUM") as psum,
        ):
            jts = []
            for k in range(8):
                t_ = jt_pool.tile([128, NRB], dt.float32r, tag=f"jt{k}")
                nc.sync.dma_start(t_[:], jt_d.ap()[k * 128:(k + 1) * 128, :])
                jts.append(t_)
            hcol = jt_pool.tile([128, 8], dt.float32, tag="hcol")
            nc.sync.dma_start(hcol[:], hcol_d.ap())
            hrow = jt_pool.tile([1, NRB], dt.float32r, tag="hrow")
            nc.sync.dma_start(hrow[:], hrow_d.ap())
            eye = jt_pool.tile([128, 128], dt.float32, tag="eye")
            nc.sync.dma_start(eye[:], eye_d.ap())
            ones_f = jt_pool.tile([1, BSH], dt.float32, tag="ones")
            nc.vector.memset(ones_f[:], 1.0)
            b_p1 = jt_pool.tile([128, 1], dt.float32, tag="bp1")
            nc.vector.memset(b_p1[:], 1.0)
            b_m1 = jt_pool.tile([128, 1], dt.float32, tag="bm1")
            nc.vector.memset(b_m1[:], -1.0)
            b_th = jt_pool.tile([128, 1], dt.float32, tag="bth")
            nc.vector.memset(b_th[:], 50.5)

            xp, vp = [], []
            for jj in range(NPAIR):
                xt = state_pool.tile([128, 1024], dt.float32r, tag=f"x{jj}")
                nc.sync.dma_start(xt[:, 0:BSH],
                                  x_d.ap()[(2 * jj) * 128:(2 * jj + 1) * 128, :])
                nc.sync.dma_start(xt[:, BSH:2 * BSH],
                                  x_d.ap()[(2 * jj + 1) * 128:(2 * jj + 2) * 128, :])
                xp.append(xt)
                vt = state_pool.tile([128, 1024], dt.float32r, tag=f"v{jj}")
                nc.sync.dma_start(vt[:, 0:BSH],
                                  v_d.ap()[(2 * jj) * 128:(2 * jj + 1) * 128, :])
                nc.sync.dma_start(vt[:, BSH:2 * BSH],
                                  v_d.ap()[(2 * jj + 1) * 128:(2 * jj + 2) * 128, :])
                vp.append(vt)

            for t in range(T):
                dgx = diag_pool.tile([128, 128], dt.float32r, tag="dgx")
                nc.vector.tensor_scalar(dgx[:], eye[:], float(cx[t]), None, Alu.mult)
                dgv = diag_pool.tile([128, 128], dt.float32r, tag="dgv")
                nc.vector.tensor_scalar(dgv[:], eye[:], float(cv[t]), None, Alu.mult)
                if E1_VARIANT == "act":
                    hb = diag_pool.tile([128, 8], dt.float32, tag="hb")
                    nc.vector.tensor_scalar(hb[:], hcol[:], float(scl_hb[t]),
                                            None, Alu.mult)

                newx, newv = [], []
                saved = []
                prev_mm = None
                for jj in range(NPAIR):
                    acc = psum.tile([128, 2 * BSH], dt.float32, tag=f"ps{jj}")
                    halves = [slice(0, BSH), slice(BSH, 2 * BSH)]
                    first_mm_of_group = None
                    last = None
                    # k-major across the two halves so late-arriving x pairs
                    # are needed as late as possible inside the group
                    kh = ([(k, h) for k in range(8) for h in range(2)]
                          if KMAJOR_HALVES else
                          [(k, h) for h in range(2) for k in range(8)])
                    for k, half in kh:
                        sl = halves[half]
                        m = 2 * jj + half
                        last = nc.tensor.matmul(
                            acc[:, sl], jts[k][:, m * 128:(m + 1) * 128],
                            xp[k // 2][:, (k % 2) * BSH:(k % 2 + 1) * BSH],
                            start=(k == 0), stop=False)
                        if first_mm_of_group is None:
                            first_mm_of_group = last
                    for half in range(2):
                        sl = halves[half]
                        last = nc.tensor.matmul(acc[:, sl], dgx[:], xp[jj][:, sl],
                                                start=False, stop=False)
                    for half in range(2):
                        sl = halves[half]
                        last = nc.tensor.matmul(acc[:, sl], dgv[:], vp[jj][:, sl],
                                                start=False, stop=True)
                    prev_mm = last

                    x = xp[jj]
                    xs = tmp_pool.tile([128, 1024], dt.float32, tag="xs")
                    xn = state_pool.tile([128, 1024], dt.float32r, tag=f"x{jj}")
                    fine = jj in FINE_PAIRS  # half-granular chain for late pairs
                    if fine:
                        for half in range(2):
                            m = 2 * jj + half
                            sl = halves[half]
                            nc.scalar.activation(xs[:, sl], acc[:, sl], Act.Identity,
                                                 bias=hb[:, m:m + 1],
                                                 scale=float(dka[t]))
                            s1 = tmp_pool.tile([128, BSH], dt.float32, tag="s1h")
                            nc.scalar.activation(s1[:], xs[:, sl], Act.Silu,
                                                 bias=b_p1[:, :])
                            s2 = tmp_pool.tile([128, BSH], dt.float32, tag="s2h")
                            nc.scalar.activation(s2[:], xs[:, sl], Act.Silu,
                                                 bias=b_m1[:, :])
                            nc.vector.scalar_tensor_tensor(
                                xn[:, sl], s1[:], -1.0, s2[:],
                                Alu.add, Alu.subtract)
                    else:
                        for half in range(2):
                            m = 2 * jj + half
                            sl = halves[half]
                            nc.scalar.activation(xs[:, sl], acc[:, sl], Act.Identity,
                                                 bias=hb[:, m:m + 1],
                                                 scale=float(dka[t]))
                        s1 = tmp_pool.tile([128, 1024], dt.float32, tag="s1")
                        nc.scalar.activation(s1[:], xs[:], Act.Silu, bias=b_p1[:, :])
                        s2 = tmp_pool.tile([128, 1024], dt.float32, tag="s2")
                        nc.scalar.activation(s2[:], xs[:], Act.Silu, bias=b_m1[:, :])
                        nc.vector.scalar_tensor_tensor(
                            xn[:], s1[:], -1.0, s2[:], Alu.add, Alu.subtract)
                    newx.append(xn)
                    saved.append((jj, x, xs, xn))

                # yv-path (deferred: not needed until late next step)
                if t < T - 1:
                    mps = []
                    if QUAD_MASK:
                        axq = []
                        mpq = []
                        for q in range(2):
                            axq_t = tmpq_pool.tile([128, 2048], dt.float32, tag=f"axq{q}")
                            axq.append(axq_t)
                            mpq_t = tmpq_pool.tile([128, 2048], dt.float32, tag=f"mpq{q}")
                            mpq.append(mpq_t)
                        for jj, x, xs, xn in saved:
                            q, qh = jj // 2, (jj % 2) * 1024
                            nc.vector.tensor_scalar(
                                axq[q][:, qh:qh + 1024].bitcast(dt.int32),
                                xn[:].bitcast(dt.int32),
                                0x7FFFFFFF, None, Alu.bitwise_and)
                        for q in range(2):
                            nc.scalar.activation(mpq[q][:], axq[q][:], Act.Tanh,
                                                 bias=b_th[:, :], scale=-50.0)
                            nc.vector.tensor_scalar(mpq[q][:], mpq[q][:], 1.0,
                                                    None, Alu.add)
                        mps = [mpq[jj // 2][:, (jj % 2) * 1024:(jj % 2 + 1) * 1024]
                               for jj in range(NPAIR)]
                    else:
                        for jj, x, xs, xn in saved:
                            axn = tmp_pool.tile([128, 1024], dt.float32, tag="axn")
                            nc.vector.tensor_scalar(
                                axn[:].bitcast(dt.int32), xn[:].bitcast(dt.int32),
                                0x7FFFFFFF, None, Alu.bitwise_and)
                            mpp = tmp_pool.tile([128, 1024], dt.float32, tag=f"mpp{jj}")
                            nc.scalar.activation(mpp[:], axn[:], Act.Tanh,
                                                 bias=b_th[:, :], scale=-50.0)
                            nc.vector.tensor_scalar(mpp[:], mpp[:], 1.0,
                                                    None, Alu.add)
                            mps.append(mpp[:])
                    for jj, x, xs, xn in saved:
                        W = tmp_pool.tile([128, 1024], dt.float32, tag="W")
                        weng = nc.gpsimd if jj in W_GPS_PAIRS else nc.vector
                        weng.tensor_tensor(
                            W[:], x[:].bitcast(dt.float32), xs[:], Alu.subtract)
                        vn = state_pool.tile([128, 1024], dt.float32r, tag=f"v{jj}")
                        eng = nc.gpsimd if jj in E9_GPS_PAIRS else nc.vector
                        eng.tensor_tensor(vn[:], mps[jj], W[:], Alu.mult)
                        newv.append(vn)
                else:
                    newv = vp
                xp, vp = newx, newv

            for jj in range(NPAIR):
                nc.sync.dma_start(out_d.ap()[(2 * jj) * 128:(2 * jj + 1) * 128, :],
                                  xp[jj][:, 0:BSH].bitcast(dt.float32))
                nc.sync.dma_start(out_d.ap()[(2 * jj + 1) * 128:(2 * jj + 2) * 128, :],
                                  xp[jj][:, BSH:2 * BSH].bitcast(dt.float32))
    nc.compile()
    return nc


def kernel(**inputs):
    H_real = np.asarray(inputs["H_real"], np.float32)
    H_imag = np.asarray(inputs["H_imag"], np.float32)
    y_real = np.asarray(inputs["y_real"], np.float32)
    y_imag = np.asarray(inputs["y_imag"], np.float32)
    Delta = np.asarray(inputs["Delta"], np.float32)
    eta = np.asarray(inputs["eta"], np.float32)
    lam = np.asarray(inputs["lam"], np.float32)
    x0 = np.asarray(inputs["x0"], np.float32)
    y0 = np.asarray(inputs["y0"], np.float32)
    nbps = int(np.asarray(inputs["nbps"]))

    J, h, c0 = _lm_setup(H_real, H_imag, y_real, y_imag, lam, nbps)

    T = Delta.shape[0]
    dk = Delta.astype(np.float64)
    a_sched = np.linspace(0.0, 1.0, T).astype(np.float32).astype(np.float64)
    eta0 = float(eta[0])
    alpha = dk * eta0 * float(c0)
    beta = dk * (1.0 - a_sched)
    dka = dk * alpha
    if E1_VARIANT == "act":
        cx = (1.0 - dk * beta) / (dk * alpha)
    else:
        cx = -beta / alpha
    cv = np.empty(T)
    cv[0] = 1.0 / alpha[0]
    for t in range(1, T):
        cv[t] = -1.0 / (2.0 * dk[t - 1] * alpha[t])
    scl_hb = dka  # bias scale for h in E1

    key = (T, E1_VARIANT, E9_GPS_PAIRS, W_GPS_PAIRS, KMAJOR_HALVES, FINE_PAIRS, QUAD_MASK,
           dka.tobytes(), cx.tobytes(), cv.tobytes())
    if key not in _BUILD_CACHE:
        _BUILD_CACHE[key] = _build(T, dka, cx, cv, scl_hb)
    nc = _BUILD_CACHE[key]

    JT = np.ascontiguousarray(J.T)
    x_init = (0.02 * (x0 - 0.5)).astype(np.float32)
    v_init = (0.02 * (y0 - 0.5)).astype(np.float32)
    hcol = np.ascontiguousarray(h.reshape(8, 128).T)  # [128, 8] per-block h
    hrow = np.ascontiguousarray(h.reshape(1, NRB))
    eye = np.eye(128, dtype=np.float32)

    in_maps = []
    for c in range(NCORES):
        sl = slice(c * BSH, (c + 1) * BSH)
        in_maps.append({
            "jt": JT,
            "x0": np.ascontiguousarray(x_init[:, sl]),
            "v0": np.ascontiguousarray(v_init[:, sl]),
            "hcol": hcol,
            "hrow": hrow,
            "eye": eye,
        })

    from concourse.bass_utils import run_bass_kernel_spmd
    res = run_bass_kernel_spmd(nc, in_maps, core_ids=list(range(NCORES)),
                               trace=False)
    out = np.concatenate([r["xout"] for r in res.results], axis=1)  # [1024, 4096]
    return np.ascontiguousarray(out.T).astype(np.float32)


# revision 16
# speedup vs baseline: 1.3400x; 1.0004x over previous
"""DU-LM-SB Trainium2 kernel.

Host: LM setup (J, h, c0) in numpy (one-time, ~0.5 GFLOP).
Device (8 NeuronCores, batch-sharded 512/core): T-step SB recurrence.

Per step t (all per-step scalars baked as immediates at build time):
    PSUM = J@x + [h] + cx*x + cv*v          (PE: fp32r matmuls, diag trick)
    xs   = dka*PSUM (+ dka*h via ACT bias)  -> x' (position update)
    s1, s2 = Silu(xs+1), Silu(xs-1)         (ACT)
    xn   = (s1 - 1) - s2                    = phi_s(x')  (DVE, f32r out)
    axn  = |xn|                             (DVE int-and)
    mp   = 1 + tanh(50.5 - 50*axn)          = 2*(1 - psi_s(xn))  (ACT + DVE)
    W    = x - xs                           = -dk*y'   (GPSIMD)
    v    = mp*W                             (DVE/GPSIMD)
with yv_t = -v_{t-1}/(2*dk_{t-1}) folded into the cv coefficient.
"""
import math

import numpy as np

NCORES = 8
NRB = 1024          # N*rb = 2*Nt*rb
BSZ = 4096
BSH = BSZ // NCORES  # 512 batch per core
NPAIR = 4            # 8 row-blocks of 128, paired into [128, 1024] tiles

# tuning flags
E1_VARIANT = "act"       # "act": ACT copy w/ h bias + carry via diag MM
                         # "stt": DVE STT w/ exact fp32 x carry + h via PE
E9_GPS_PAIRS = ()    # which pairs run the final v=mp*W product on GPSIMD
W_GPS_PAIRS = ()  # W engine per pair
KMAJOR_HALVES = False     # interleave the two psum halves k-major in PE order
FINE_PAIRS = ()      # pairs with half-granular xn chains
QUAD_MASK = False         # quad-granular abs/tanh/mp in yv path

_BUILD_CACHE = {}


def _lm_setup(H_real, H_imag, y_real, y_imag, lam, nbps):
    M = 2 ** nbps
    Nr, Nt = H_real.shape
    N = 2 * Nt
    rb = nbps // 2
    qam_var = 2.0 * (M - 1) / 3.0
    I_N = np.eye(N, dtype=np.float32)
    w = (2.0 ** (rb - 1 - np.arange(rb))).astype(np.float32)
    Tm = (w[:, None, None] * I_N[None]).reshape(-1, N).T
    H_tilde = np.block([[H_real, -H_imag], [H_imag, H_real]]).astype(np.float32)
    y_tilde = np.concatenate([y_real, y_imag], axis=0).astype(np.float32)
    G = (H_tilde @ H_tilde.T + lam * I_N).astype(np.float32)
    U = (np.linalg.inv(G) / lam).astype(np.float32)
    HT = (H_tilde @ Tm).astype(np.float32)
    J = (-(HT.T @ (U @ HT)) * np.float32(2.0 / qam_var)).astype(np.float32)
    J = (J * (1.0 - np.eye(J.shape[0], dtype=J.dtype))).astype(np.float32)
    z = ((y_tilde - HT @ np.ones((N * rb, 1), np.float32)
          + np.float32(math.sqrt(M) - 1.0) * (H_tilde @ np.ones((N, 1), np.float32)))
         / np.float32(math.sqrt(qam_var))).astype(np.float32)
    h = (2.0 * HT.T @ (U @ z)).astype(np.float32)
    c0 = np.float32(0.5 * math.sqrt(J.shape[0] - 1) / np.linalg.norm(J))
    return J, h, c0


def _build(T, dka, cx, cv, scl_hb):
    """Build + bacc-compile the per-core Bass program (all cores identical)."""
    import concourse.bacc as bacc
    import concourse.mybir as mybir
    import concourse.tile as tile
    from concourse.tile import add_dep_helper

    dt = mybir.dt
    Alu = mybir.AluOpType
    Act = mybir.ActivationFunctionType

    nc = bacc.Bacc(None, target_bir_lowering=False, debug=False)
    jt_d = nc.dram_tensor("jt", [NRB, NRB], dt.float32r, kind="ExternalInput")
    x_d = nc.dram_tensor("x0", [NRB, BSH], dt.float32r, kind="ExternalInput")
    v_d = nc.dram_tensor("v0", [NRB, BSH], dt.float32r, kind="ExternalInput")
    hcol_d = nc.dram_tensor("hcol", [128, 8], dt.float32, kind="ExternalInput")
    hrow_d = nc.dram_tensor("hrow", [1, NRB], dt.float32r, kind="ExternalInput")
    eye_d = nc.dram_tensor("eye", [128, 128], dt.float32, kind="ExternalInput")
    out_d = nc.dram_tensor("xout", [NRB, BSH], dt.float32, kind="ExternalOutput")

    with tile.TileContext(nc) as tc:
        with (
            tc.tile_pool(name="jt", bufs=1) as jt_pool,
            tc.tile_pool(name="state", bufs=2) as state_pool,
            tc.tile_pool(name="tmp", bufs=2) as tmp_pool,
            tc.tile_pool(name="tmpq", bufs=1) as tmpq_pool,
            tc.tile_pool(name="diag", bufs=2) as diag_pool,
            tc.tile_pool(name="ps", bufs=1, space="PSUM") as psum,
        ):
            jts = []
            for k in range(8):
                t_ = jt_pool.tile([128, NRB], dt.float32r, tag=f"jt{k}")
                nc.sync.dma_start(t_[:], jt_d.ap()[k * 128:(k + 1) * 128, :])
                jts.append(t_)
            hcol = jt_pool.tile([128, 8], dt.float32, tag="hcol")
            nc.sync.dma_start(hcol[:], hcol_d.ap())
            hrow = jt_pool.tile([1, NRB], dt.float32r, tag="hrow")
            nc.sync.dma_start(hrow[:], hrow_d.ap())
            eye = jt_pool.tile([128, 128], dt.float32, tag="eye")
            nc.sync.dma_start(eye[:], eye_d.ap())
            ones_f = jt_pool.tile([1, BSH], dt.float32, tag="ones")
            nc.vector.memset(ones_f[:], 1.0)
            b_p1 = jt_pool.tile([128, 1], dt.float32, tag="bp1")
            nc.vector.memset(b_p1[:], 1.0)
            b_m1 = jt_pool.tile([128, 1], dt.float32, tag="bm1")
            nc.vector.memset(b_m1[:], -1.0)
            b_th = jt_pool.tile([128, 1], dt.float32, tag="bth")
            nc.vector.memset(b_th[:], 50.5)

            xp, vp = [], []
            for jj in range(NPAIR):
                xt = state_pool.tile([128, 1024], dt.float32r, tag=f"x{jj}")
                nc.sync.dma_start(xt[:, 0:BSH],
                                  x_d.ap()[(2 * jj) * 128:(2 * jj + 1) * 128, :])
                nc.sync.dma_start(xt[:, BSH:2 * BSH],
                                  x_d.ap()[(2 * jj + 1) * 128:(2 * jj + 2) * 128, :])
                xp.append(xt)
                vt = state_pool.tile([128, 1024], dt.float32r, tag=f"v{jj}")
                nc.sync.dma_start(vt[:, 0:BSH],
                                  v_d.ap()[(2 * jj) * 128:(2 * jj + 1) * 128, :])
                nc.sync.dma_start(vt[:, BSH:2 * BSH],
                                  v_d.ap()[(2 * jj + 1) * 128:(2 * jj + 2) * 128, :])
                vp.append(vt)

            for t in range(T):
                dgx = diag_pool.tile([128, 128], dt.float32r, tag="dgx")
                nc.vector.tensor_scalar(dgx[:], eye[:], float(cx[t]), None, Alu.mult)
                dgv = diag_pool.tile([128, 128], dt.float32r, tag="dgv")
                nc.vector.tensor_scalar(dgv[:], eye[:], float(cv[t]), None, Alu.mult)
                if E1_VARIANT == "act":
                    hb = diag_pool.tile([128, 8], dt.float32, tag="hb")
                    nc.vector.tensor_scalar(hb[:], hcol[:], float(scl_hb[t]),
                                            None, Alu.mult)

                newx, newv = [], []
                saved = []
                prev_mm = None
                for jj in range(NPAIR):
                    acc = psum.tile([128, 2 * BSH], dt.float32, tag=f"ps{jj}")
                    halves = [slice(0, BSH), slice(BSH, 2 * BSH)]
                    first_mm_of_group = None
                    last = None
                    # k-major across the two halves so late-arriving x pairs
                    # are needed as late as possible inside the group
                    kh = ([(k, h) for k in range(8) for h in range(2)]
                          if KMAJOR_HALVES else
                          [(k, h) for h in range(2) for k in range(8)])
                    for k, half in kh:
                        sl = halves[half]
                        m = 2 * jj + half
                        last = nc.tensor.matmul(
                            acc[:, sl], jts[k][:, m * 128:(m + 1) * 128],
                            xp[k // 2][:, (k % 2) * BSH:(k % 2 + 1) * BSH],
                            start=(k == 0), stop=False)
                        if first_mm_of_group is None:
                            first_mm_of_group = last
                    for half in range(2):
                        sl = halves[half]
                        last = nc.tensor.matmul(acc[:, sl], dgx[:], xp[jj][:, sl],
                                                start=False, stop=False)
                    for half in range(2):
                        sl = halves[half]
                        last = nc.tensor.matmul(acc[:, sl], dgv[:], vp[jj][:, sl],
                                                start=False, stop=True)
                    prev_mm = last

                    x = xp[jj]
                    xs = tmp_pool.tile([128, 1024], dt.float32, tag="xs")
                    xn = state_pool.tile([128, 1024], dt.float32r, tag=f"x{jj}")
                    fine = jj in FINE_PAIRS  # half-granular chain for late pairs
                    if fine:
                        for half in range(2):
                            m = 2 * jj + half
                            sl = halves[half]
                            nc.scalar.activation(xs[:, sl], acc[:, sl], Act.Identity,
                                                 bias=hb[:, m:m + 1],
                                                 scale=float(dka[t]))
                            s1 = tmp_pool.tile([128, BSH], dt.float32, tag="s1h")
                            nc.scalar.activation(s1[:], xs[:, sl], Act.Silu,
                                                 bias=b_p1[:, :])
                            s2 = tmp_pool.tile([128, BSH], dt.float32, tag="s2h")
                            nc.scalar.activation(s2[:], xs[:, sl], Act.Silu,
                                                 bias=b_m1[:, :])
                            nc.vector.scalar_tensor_tensor(
                                xn[:, sl], s1[:], -1.0, s2[:],
                                Alu.add, Alu.subtract)
                    else:
                        for half in range(2):
                            m = 2 * jj + half
                            sl = halves[half]
                            nc.scalar.activation(xs[:, sl], acc[:, sl], Act.Identity,
                                                 bias=hb[:, m:m + 1],
                                                 scale=float(dka[t]))
                        s1 = tmp_pool.tile([128, 1024], dt.float32, tag="s1")
                        nc.scalar.activation(s1[:], xs[:], Act.Silu, bias=b_p1[:, :])
                        s2 = tmp_pool.tile([128, 1024], dt.float32, tag="s2")
                        nc.scalar.activation(s2[:], xs[:], Act.Silu, bias=b_m1[:, :])
                        nc.vector.scalar_tensor_tensor(
                            xn[:], s1[:], -1.0, s2[:], Alu.add, Alu.subtract)
                    newx.append(xn)
                    saved.append((jj, x, xs, xn))

                # yv-path (deferred: not needed until late next step)
                if t < T - 1:
                    mps = []
                    if QUAD_MASK:
                        axq = []
                        mpq = []
                        for q in range(2):
                            axq_t = tmpq_pool.tile([128, 2048], dt.float32, tag=f"axq{q}")
                            axq.append(axq_t)
                            mpq_t = tmpq_pool.tile([128, 2048], dt.float32, tag=f"mpq{q}")
                            mpq.append(mpq_t)
                        for jj, x, xs, xn in saved:
                            q, qh = jj // 2, (jj % 2) * 1024
                            nc.vector.tensor_scalar(
                                axq[q][:, qh:qh + 1024].bitcast(dt.int32),
                                xn[:].bitcast(dt.int32),
                                0x7FFFFFFF, None, Alu.bitwise_and)
                        for q in range(2):
                            nc.scalar.activation(mpq[q][:], axq[q][:], Act.Tanh,
                                                 bias=b_th[:, :], scale=-50.0)
                            nc.vector.tensor_scalar(mpq[q][:], mpq[q][:], 1.0,
                                                    None, Alu.add)
                        mps = [mpq[jj // 2][:, (jj % 2) * 1024:(jj % 2 + 1) * 1024]
                               for jj in range(NPAIR)]
                    else:
                        for jj, x, xs, xn in saved:
                            axn = tmp_pool.tile([128, 1024], dt.float32, tag="axn")
                            nc.vector.tensor_scalar(
                                axn[:].bitcast(dt.int32), xn[:].bitcast(dt.int32),
                                0x7FFFFFFF, None, Alu.bitwise_and)
                            mpp = tmp_pool.tile([128, 1024], dt.float32, tag=f"mpp{jj}")
                            nc.scalar.activation(mpp[:], axn[:], Act.Tanh,
                                                 bias=b_th[:, :], scale=-50.0)
                            nc.vector.tensor_scalar(mpp[:], mpp[:], 1.0,
                                                    None, Alu.add)
                            mps.append(mpp[:])
                    for jj, x, xs, xn in saved:
                        W = tmp_pool.tile([128, 1024], dt.float32, tag="W")
                        weng = nc.gpsimd if jj in W_GPS_PAIRS else nc.vector
                        weng.tensor_tensor(
                            W[:], x[:].bitcast(dt.float32), xs[:], Alu.subtract)
                        vn = state_pool.tile([128, 1024], dt.float32r, tag=f"v{jj}")
                        eng = nc.gpsimd if jj in E9_GPS_PAIRS else nc.vector
                        eng.tensor_tensor(vn[:], mps[jj], W[:], Alu.mult)
                        newv.append(vn)
                else:
                    newv = vp
                xp, vp = newx, newv

            for jj in range(NPAIR):
                nc.sync.dma_start(out_d.ap()[(2 * jj) * 128:(2 * jj + 1) * 128, :],
                                  xp[jj][:, 0:BSH].bitcast(dt.float32))
                nc.sync.dma_start(out_d.ap()[(2 * jj + 1) * 128:(2 * jj + 2) * 128, :],
                                  xp[jj][:, BSH:2 * BSH].bitcast(dt.float32))
    nc.compile()
    return nc


def kernel(**inputs):
    H_real = np.asarray(inputs["H_real"], np.float32)
    H_imag = np.asarray(inputs["H_imag"], np.float32)
    y_real = np.asarray(inputs["y_real"], np.float32)
    y_imag = np.asarray(inputs["y_imag"], np.float32)
    Delta = np.asarray(inputs["Delta"], np.float32)
    eta = np.asarray(inputs["eta"], np.float32)
    lam = np.asarray(inputs["lam"], np.float32)
    x0 = np.asarray(inputs["x0"], np.float32)
    y0 = np.asarray(inputs["y0"], np.float32)
    nbps = int(np.asarray(inputs["nbps"]))

    J, h, c0 = _lm_setup(H_real, H_imag, y_real, y_imag, lam, nbps)

    T = Delta.shape[0]
    dk = Delta.astype(np.float64)
    a_sched = np.linspace(0.0, 1.0, T).astype(np.float32).astype(np.float64)
    eta0 = float(eta[0])
    alpha = dk * eta0 * float(c0)
    beta = dk * (1.0 - a_sched)
    dka = dk * alpha
    if E1_VARIANT == "act":
        cx = (1.0 - dk * beta) / (dk * alpha)
    else:
        cx = -beta / alpha
    cv = np.empty(T)
    cv[0] = 1.0 / alpha[0]
    for t in range(1, T):
        cv[t] = -1.0 / (2.0 * dk[t - 1] * alpha[t])
    scl_hb = dka  # bias scale for h in E1

    key = (T, E1_VARIANT, E9_GPS_PAIRS, W_GPS_PAIRS, KMAJOR_HALVES, FINE_PAIRS, QUAD_MASK,
           dka.tobytes(), cx.tobytes(), cv.tobytes())
    if key not in _BUILD_CACHE:
        _BUILD_CACHE[key] = _build(T, dka, cx, cv, scl_hb)
    nc = _BUILD_CACHE[key]

    JT = np.ascontiguousarray(J.T)
    x_init = (0.02 * (x0 - 0.5)).astype(np.float32)
    v_init = (0.02 * (y0 - 0.5)).astype(np.float32)
    hcol = np.ascontiguousarray(h.reshape(8, 128).T)  # [128, 8] per-block h
    hrow = np.ascontiguousarray(h.reshape(1, NRB))
    eye = np.eye(128, dtype=np.float32)

    in_maps = []
    for c in range(NCORES):
        sl = slice(c * BSH, (c + 1) * BSH)
        in_maps.append({
            "jt": JT,
            "x0": np.ascontiguousarray(x_init[:, sl]),
            "v0": np.ascontiguousarray(v_init[:, sl]),
            "hcol": hcol,
            "hrow": hrow,
            "eye": eye,
        })

    from concourse.bass_utils import run_bass_kernel_spmd
    res = run_bass_kernel_spmd(nc, in_maps, core_ids=list(range(NCORES)),
                               trace=False)
    out = np.concatenate([r["xout"] for r in res.results], axis=1)  # [1024, 4096]
    return np.ascontiguousarray(out.T).astype(np.float32)
